# revision 1
# baseline (speedup 1.0000x reference)
"""Trainium2 Bass kernel for nn_MixedRepeatHeads.

Computation (full shapes):
  proj[h,b,k] = einsum(x[b,d], proj_w[h,k,d]) + proj_b[h,k]
  w = mix_w[:, index]; bb = mix_b[:, index]
  decay = clip(decay_value, 0.9, 1.0) ** (1/8)
  coef[h] = w*decay (h<8) else decay
  hidden[b, h*256+k] = w[h]*proj[h,b,k] + coef[h]*cache[h,b,k] + bb[h]
  out = hidden @ out_w.T + out_b                     # [8192, 4096]

Strategy: data-parallel over batch across 8 cores (1024 rows each).
All per-head scalars are folded on the host:
  PT[d, i=h*256+k] = w[h] * proj_w[h,k,d]           # stage-A weights
  B2[i, j]         = out_w[j, i]                    # stage-B weights
  cacheF[i, b]     = coef[h]*cache[h,b,k] + (w[h]*proj_b[h,k] + bb[h])
  xT[d, b]         = x.T
Per core (b = 1024, processed in two halves of 512):
  stage A: hiddenT[i, b] = sum_d PT[d,i]^T @ xT[d,b] + cacheF[i,b]   (f32r MMs)
  stage B: outT[j, b]    = sum_i B2[i,j]^T @ hiddenT[i,b] + out_b[j] (f32r MMs)
Device output is outT [4096, 1024] per core; host transposes and concatenates.
"""

import sys

if "/opt/trn_rl_repo" not in sys.path:
    sys.path.insert(0, "/opt/trn_rl_repo")

import numpy as np

import bass_rust
import concourse.bass as bass
import concourse.tile as tile
from concourse import mybir
from concourse.bass_utils import run_bass_kernel_spmd
from concourse.vector_clock import ScopedClock

# ---------------------------------------------------------------- constants
N_HEADS = 16
HIDDEN = 256
DIM = 4096  # d == i == j == 4096
BATCH = 8192
DECAY_CONSTANT = 8
N_CORES = 8
BC = BATCH // N_CORES  # 1024 batch rows per core
HALF = BC // 2  # 512
P = 128
DT = DIM // P  # 32 tiles along any 4096 dim

F32 = mybir.dt.float32
F32R = mybir.dt.float32r

# ------------------------------------------------- walrus wait legalization
# This walrus build supports only ONE sync-wait command per instruction.
MAXW = 1


class SafeTileContext(tile.TileContext):
    def _split_waits_in_ordered(self, ordered):
        nc = self.nc
        for _bb_name, insts in ordered.items():
            new_list = []
            changed = False
            for inst in insts:
                si = inst.sync_info
                if si is not None and len(si.on_wait) > MAXW:
                    waits = list(si.on_wait)
                    ups = list(si.on_update)
                    head, tail = waits[:-MAXW], waits[-MAXW:]
                    for w in head:
                        nop = mybir.InstNoOp(
                            name=nc.get_next_instruction_name(),
                            engine=inst.engine,
                            ins=[],
                            outs=[],
                            sync_info=bass_rust.SyncInfo(on_wait=[w], on_update=[]),
                            bass_nofuse=True,
                        )
                        nc.register_instruction(nop, overwrite=True)
                        new_list.append(nop)
                    inst.sync_info = bass_rust.SyncInfo(on_wait=tail, on_update=ups)
                    changed = True
                new_list.append(inst)
            if changed:
                insts[:] = new_list
        return ordered

    def _lower_ordered_insts(self, ordered):
        self._split_waits_in_ordered(ordered)
        return super()._lower_ordered_insts(ordered)

    def _drain_and_barrier(self, tick_clock, wait_clock):
        probe = self.nc.sync.nop(nofuse=True)
        wait_clock.add_sem_waits(
            probe.ins, ScopedClock({None: tick_clock.global_clock})
        )
        si = probe.ins.sync_info
        waits = list(si.on_wait) if si is not None else []
        upd = list(si.on_update) if si is not None else []
        probe.ins.sync_info = bass_rust.SyncInfo(on_wait=waits[:MAXW], on_update=upd)
        for i in range(MAXW, len(waits), MAXW):
            n = self.nc.sync.nop(nofuse=True)
            n.ins.sync_info = bass_rust.SyncInfo(
                on_wait=waits[i : i + MAXW], on_update=[]
            )

        self.nc.sync.drain()

        self.nc.all_engine_barrier()
        assert self.sems is not None
        popped = self.nc._tile_sem_poison_stack.pop()
        assert popped is self._sem_poison
        self.nc.clear_and_free_semaphores(list(self.sems.allocated().values()))
        self.nc.all_engine_barrier()


# ------------------------------------------------------------ kernel build
def build_kernel(probe_reuse_weights=False, loop_reps=None, deep_bufs=False):
    """Per-core program. DRAM params:
      xT  [DIM, BC]  f32r : x.T slice for this core
      cf  [DIM, BC]  f32  : cacheF slice (cache term + bias, i-major)
      pt  [DIM, DIM] f32r : PT  (stage-A weights, [d, i])
      b2  [DIM, DIM] f32r : B2  (stage-B weights, [i, j])
      ob  [P, DT]    f32  : out_b strided per partition: ob[p, jt] = out_b[jt*128+p]
      outT [DIM, BC] f32  : output, transposed (j-major)
    """
    nc = bass.Bass()
    xT = nc.declare_dram_parameter("xT", [DIM, BC], F32R, isOutput=False)
    cf = nc.declare_dram_parameter("cf", [DIM, BC], F32, isOutput=False)
    pt = nc.declare_dram_parameter("pt", [DT, P, DT, P], F32R, isOutput=False)
    b2 = nc.declare_dram_parameter("b2", [DT, P, DT, P], F32R, isOutput=False)
    ob = nc.declare_dram_parameter("ob", [P, DT], F32, isOutput=False)
    outT = nc.declare_dram_parameter("outT", [DIM, BC], F32, isOutput=True)

    # [d, b] -> [p, do, b] view with d-inner on partitions
    xT_v = xT.rearrange("(o p) b -> p o b", p=P)

    with SafeTileContext(nc) as tc:
        with (
            tc.tile_pool(name="xpool", bufs=1) as xpool,
            tc.tile_pool(name="hpool", bufs=1) as hpool,
            tc.tile_pool(name="spool", bufs=3) as spool,
            tc.tile_pool(name="cfpool", bufs=3 if deep_bufs else 2) as cfpool,
            tc.tile_pool(name="opool", bufs=4 if deep_bufs else 3) as opool,
            tc.tile_pool(name="obpool", bufs=1) as obpool,
            tc.tile_pool(
                name="pspool", bufs=6 if deep_bufs else 4, space="PSUM"
            ) as pspool,
        ):
            ob_t = obpool.tile([P, DT], F32)
            nc.sync.dma_start(ob_t[:], ob[:])

            reused_blk = None
            if probe_reuse_weights:
                reused_blk = spool.tile([P, DT, P], F32R, tag="wblk")
                nc.sync.dma_start(reused_blk[:], pt[0])

            import contextlib

            loop_cm = (
                tc.For_i(0, loop_reps, 1)
                if loop_reps is not None
                else contextlib.nullcontext()
            )
            with loop_cm:
              for half in range(2):
                  bsl = slice(half * HALF, (half + 1) * HALF)
                  XCH = 4  # x chunks along d-outer: first MM waits only 2 MiB
                  DCH = DT // XCH
                  x_ch = []
                  for xc in range(XCH):
                      t = xpool.tile([P, DCH, HALF], F32R, tag=f"x{xc}")
                      nc.sync.dma_start(
                          t[:], xT_v[:, xc * DCH : (xc + 1) * DCH, bsl]
                      )
                      x_ch.append(t)
                  h_t = hpool.tile([P, DT, HALF], F32R, tag="h")

                  # stage A: hiddenT[i, b] over i-tiles
                  for it in range(DT):
                      if probe_reuse_weights:
                          blk = reused_blk
                      else:
                          blk = spool.tile([P, DT, P], F32R, tag="wblk")
                          nc.sync.dma_start(blk[:], pt[it])
                      ps = pspool.tile([P, HALF], F32, tag="ps")
                      for dt_ in range(DT):
                          nc.tensor.matmul(
                              ps[:],
                              blk[:, dt_, :],
                              x_ch[dt_ // DCH][:, dt_ % DCH, :],
                              start=(dt_ == 0),
                              stop=(dt_ == DT - 1),
                          )
                      cf_t = cfpool.tile([P, HALF], F32, tag="cf")
                      nc.sync.dma_start(cf_t[:], cf[it * P : (it + 1) * P, bsl])
                      nc.vector.tensor_tensor(
                          h_t[:, it, :], ps[:], cf_t[:], mybir.AluOpType.add
                      )

                  # stage B: outT[j, b] over j-tiles
                  for jt in range(DT):
                      if probe_reuse_weights:
                          blk = reused_blk
                      else:
                          blk = spool.tile([P, DT, P], F32R, tag="wblk")
                          nc.sync.dma_start(blk[:], b2[jt])
                      ps = pspool.tile([P, HALF], F32, tag="ps")
                      for io in range(DT):
                          nc.tensor.matmul(
                              ps[:],
                              blk[:, io, :],
                              h_t[:, io, :],
                              start=(io == 0),
                              stop=(io == DT - 1),
                          )
                      o_t = opool.tile([P, HALF], F32, tag="o")
                      nc.vector.tensor_tensor(
                          o_t[:],
                          ps[:],
                          ob_t[:, jt : jt + 1].to_broadcast((P, HALF)),
                          mybir.AluOpType.add,
                      )
                      nc.sync.dma_start(outT[jt * P : (jt + 1) * P, bsl], o_t[:])

    return nc


# ------------------------------------------------------------- host helpers
def _host_prepare(inputs):
    x = np.asarray(inputs["x"], dtype=np.float32)
    proj_w = np.asarray(inputs["proj_w"], dtype=np.float32)
    proj_b = np.asarray(inputs["proj_b"], dtype=np.float32)
    mix_w = np.asarray(inputs["mix_w"], dtype=np.float32)
    mix_b = np.asarray(inputs["mix_b"], dtype=np.float32)
    decay_value = np.asarray(inputs["decay_value"], dtype=np.float32)
    cache = np.asarray(inputs["cache"], dtype=np.float32)
    out_w = np.asarray(inputs["out_w"], dtype=np.float32)
    out_b = np.asarray(inputs["out_b"], dtype=np.float32)
    idx = int(np.asarray(inputs["index"]))

    w = mix_w[:, idx]  # [16]
    bb = mix_b[:, idx]  # [16]
    decay = np.clip(decay_value, 0.9, 1.0) ** np.float32(1.0 / DECAY_CONSTANT)
    is_col = np.arange(N_HEADS) < (N_HEADS // 2)
    coef = np.where(is_col, w * decay, decay).astype(np.float32)  # [16]

    # PT[d, i] = w[h] * proj_w[h, k, d], pre-tiled to [it, p, do, ii] so each
    # weight block for one i-tile is one contiguous 1 MiB DMA.
    pw = (proj_w * w[:, None, None]).reshape(DIM, DIM)  # [i, d]
    PT = pw.T.reshape(DT, P, DT, P).transpose(2, 1, 0, 3)  # [it, p(d), do, ii]
    PT = np.ascontiguousarray(PT)

    B2 = out_w.T.reshape(DT, P, DT, P).transpose(2, 1, 0, 3)  # [jt, p(i), io, jj]
    B2 = np.ascontiguousarray(B2)

    bias_hk = w[:, None] * proj_b + bb[:, None]  # [16, 256]
    cacheF = coef[:, None, None] * cache + bias_hk[:, None, :]  # [h, b, k]
    cacheF = np.ascontiguousarray(
        cacheF.transpose(0, 2, 1).reshape(DIM, BATCH)
    )  # [i, b]

    xT = np.ascontiguousarray(x.T)  # [d, b]

    obT = np.ascontiguousarray(out_b.reshape(DT, P).T)  # [P, DT]

    in_maps = []
    for c in range(N_CORES):
        bsl = slice(c * BC, (c + 1) * BC)
        in_maps.append(
            {
                "xT": np.ascontiguousarray(xT[:, bsl]),
                "cf": np.ascontiguousarray(cacheF[:, bsl]),
                "pt": PT,
                "b2": B2,
                "ob": obT,
            }
        )
    return in_maps


def _assemble(results):
    # results: list per core of {"outT": [DIM, BC]}
    out = np.empty((BATCH, DIM), dtype=np.float32)
    for c in range(N_CORES):
        out[c * BC : (c + 1) * BC] = results[c]["outT"].T
    return out


_NC_CACHE = None


def _get_nc():
    global _NC_CACHE
    if _NC_CACHE is None:
        _NC_CACHE = build_kernel()
    return _NC_CACHE


def kernel(**inputs) -> np.ndarray:
    in_maps = _host_prepare(inputs)
    nc = _get_nc()
    res = run_bass_kernel_spmd(nc, in_maps, list(range(N_CORES)))
    return _assemble(res.results)


if __name__ == "__main__":
    # quick self-run with random data of the right shapes
    rng = np.random.default_rng(0)
    ins = {
        "x": rng.standard_normal((BATCH, DIM), dtype=np.float32),
        "proj_w": rng.standard_normal((N_HEADS, HIDDEN, DIM), dtype=np.float32) * 0.02,
        "proj_b": rng.standard_normal((N_HEADS, HIDDEN), dtype=np.float32) * 0.02,
        "mix_w": rng.standard_normal((N_HEADS, 4096), dtype=np.float32) * 0.02 + 1.0,
        "mix_b": rng.standard_normal((N_HEADS, 4096), dtype=np.float32) * 0.02,
        "decay_value": rng.uniform(0.85, 1.05, size=(N_HEADS,)).astype(np.float32),
        "cache": rng.standard_normal((N_HEADS, BATCH, HIDDEN), dtype=np.float32),
        "out_w": rng.standard_normal((DIM, DIM), dtype=np.float32) * 0.02,
        "out_b": rng.standard_normal((DIM,), dtype=np.float32) * 0.02,
        "index": 1000,
    }
    out = kernel(**ins)
    print("out", out.shape, out.dtype, float(np.abs(out).mean()))



# revision 2
# speedup vs baseline: 1.0275x; 1.0275x over previous
"""Trainium2 Bass kernel for nn_MixedRepeatHeads.

Computation (full shapes):
  proj[h,b,k] = einsum(x[b,d], proj_w[h,k,d]) + proj_b[h,k]
  w = mix_w[:, index]; bb = mix_b[:, index]
  decay = clip(decay_value, 0.9, 1.0) ** (1/8)
  coef[h] = w*decay (h<8) else decay
  hidden[b, h*256+k] = w[h]*proj[h,b,k] + coef[h]*cache[h,b,k] + bb[h]
  out = hidden @ out_w.T + out_b                     # [8192, 4096]

Algebraic refold (all on host, cheap):
  out = x @ C.T + (coef*cache) @ out_w.T + ob2
    C[j,d]  = sum_i out_w[j,i] * (w*proj_w)[i,d]    (one 4096^3 host GEMM)
    ob2[j]  = out_b[j] + sum_i out_w[j,i]*(w[h]*proj_b + bb)[i]
  => single GEMM with K=8192:  out = A @ Wcat + ob2
    A    = [x | (coef*cache) as (b, i)]             # [8192, 8192]
    Wcat = [C.T ; out_w.T]                          # [8192, 4096]

Device strategy: data-parallel over batch across 8 cores (1024 rows each),
all-bf16 matmuls (PE runs bf16 at 1 row/cycle and its weight loads are
hoisted, unlike f32r which must self-load serially).  Per core:
  outT[j, 1024] = sum_kt Wcat_tile[kt]^T @ AT_tile[kt]  (+ ob2, vector add)
Batch is processed as two 512-column halves, each a full pass over j, so
the next half's activation DMA overlaps the current half's matmuls.
Device output is outT [4096, 1024] per core; host transposes and concats.
"""

import sys

if "/opt/trn_rl_repo" not in sys.path:
    sys.path.insert(0, "/opt/trn_rl_repo")

import numpy as np
import ml_dtypes

import bass_rust
import concourse.bass as bass
import concourse.tile as tile
from concourse import mybir
from concourse.bass_utils import run_bass_kernel_spmd
from concourse.vector_clock import ScopedClock

# ---------------------------------------------------------------- constants
N_HEADS = 16
HIDDEN = 256
DIM = 4096
BATCH = 8192
DECAY_CONSTANT = 8
N_CORES = 8
BC = BATCH // N_CORES  # 1024 batch rows per core
HALF = BC // 2  # 512
P = 128
KDIM = 2 * DIM  # 8192 contraction (x ++ cache)
KT = KDIM // P  # 64 k-tiles
JT = DIM // P  # 32 j-tiles

F32 = mybir.dt.float32
F32R = mybir.dt.float32r
BF16 = mybir.dt.bfloat16
NP_BF16 = ml_dtypes.bfloat16

# ------------------------------------------------- walrus wait legalization
# This walrus build supports only ONE sync-wait command per instruction.
MAXW = 1


class SafeTileContext(tile.TileContext):
    def _split_waits_in_ordered(self, ordered):
        nc = self.nc
        for _bb_name, insts in ordered.items():
            new_list = []
            changed = False
            for inst in insts:
                si = inst.sync_info
                if si is not None and len(si.on_wait) > MAXW:
                    waits = list(si.on_wait)
                    ups = list(si.on_update)
                    head, tail = waits[:-MAXW], waits[-MAXW:]
                    for w in head:
                        nop = mybir.InstNoOp(
                            name=nc.get_next_instruction_name(),
                            engine=inst.engine,
                            ins=[],
                            outs=[],
                            sync_info=bass_rust.SyncInfo(on_wait=[w], on_update=[]),
                            bass_nofuse=True,
                        )
                        nc.register_instruction(nop, overwrite=True)
                        new_list.append(nop)
                    inst.sync_info = bass_rust.SyncInfo(on_wait=tail, on_update=ups)
                    changed = True
                new_list.append(inst)
            if changed:
                insts[:] = new_list
        return ordered

    def _lower_ordered_insts(self, ordered):
        self._split_waits_in_ordered(ordered)
        return super()._lower_ordered_insts(ordered)

    def _drain_and_barrier(self, tick_clock, wait_clock):
        probe = self.nc.sync.nop(nofuse=True)
        wait_clock.add_sem_waits(
            probe.ins, ScopedClock({None: tick_clock.global_clock})
        )
        si = probe.ins.sync_info
        waits = list(si.on_wait) if si is not None else []
        upd = list(si.on_update) if si is not None else []
        probe.ins.sync_info = bass_rust.SyncInfo(on_wait=waits[:MAXW], on_update=upd)
        for i in range(MAXW, len(waits), MAXW):
            n = self.nc.sync.nop(nofuse=True)
            n.ins.sync_info = bass_rust.SyncInfo(
                on_wait=waits[i : i + MAXW], on_update=[]
            )

        self.nc.sync.drain()

        self.nc.all_engine_barrier()
        assert self.sems is not None
        popped = self.nc._tile_sem_poison_stack.pop()
        assert popped is self._sem_poison
        self.nc.clear_and_free_semaphores(list(self.sems.allocated().values()))
        self.nc.all_engine_barrier()


# ------------------------------------------------------------ kernel build
def build_kernel(loop_reps=None):
    """Per-core program. DRAM params:
      a    [KT, P, BC]      bf16 : AT tiles, a[kt, p, b] = A[b, kt*128+p]
      wt   [JT, P, KT, P]   bf16 : Wcat tiles, wt[jt, p, kt, j]
                                   = Wcat[kt*128+p, jt*128+j]
      ob   [P, JT]          f32  : ob2 strided: ob[p, jt] = ob2[jt*128+p]
      outT [DIM, BC]        f32  : output, transposed (j-major)
    """
    nc = bass.Bass()
    a = nc.declare_dram_parameter("a", [KT, P, BC], BF16, isOutput=False)
    wt = nc.declare_dram_parameter("wt", [JT, P, KT, P], BF16, isOutput=False)
    ob = nc.declare_dram_parameter("ob", [P, JT], F32, isOutput=False)
    outT = nc.declare_dram_parameter("outT", [DIM, BC], F32, isOutput=True)

    a_v = a.rearrange("t p b -> p t b")  # [P, KT, BC]

    ACH = 8  # a-half DMA chunks (8 kt per chunk)
    KCH = KT // ACH

    with SafeTileContext(nc) as tc:
        with (
            tc.tile_pool(name="a0pool", bufs=1) as a0pool,
            tc.tile_pool(name="a1pool", bufs=1) as a1pool,
            tc.tile_pool(name="wpool", bufs=3) as wpool,
            tc.tile_pool(name="opool", bufs=3) as opool,
            tc.tile_pool(name="obpool", bufs=1) as obpool,
            tc.tile_pool(name="pspool", bufs=4, space="PSUM") as pspool,
        ):
            ob_t = obpool.tile([P, JT], F32)
            nc.sync.dma_start(ob_t[:], ob[:])

            import contextlib

            loop_cm = (
                tc.For_i(0, loop_reps, 1)
                if loop_reps is not None
                else contextlib.nullcontext()
            )
            with loop_cm:
                for half in range(2):
                    bsl = slice(half * HALF, (half + 1) * HALF)
                    apool = a0pool if half == 0 else a1pool
                    a_t = apool.tile([P, KT, HALF], BF16, tag=f"a{half}")
                    for c in range(ACH):
                        ksl = slice(c * KCH, (c + 1) * KCH)
                        nc.sync.dma_start(a_t[:, ksl, :], a_v[:, ksl, bsl])

                    for jt in range(JT):
                        wblk = wpool.tile([P, KT, P], BF16, tag="wblk")
                        nc.sync.dma_start(wblk[:], wt[jt])
                        ps = pspool.tile([P, HALF], F32, tag="ps")
                        for kt in range(KT):
                            nc.tensor.matmul(
                                ps[:],
                                wblk[:, kt, :],
                                a_t[:, kt, :],
                                start=(kt == 0),
                                stop=(kt == KT - 1),
                            )
                        o_t = opool.tile([P, HALF], F32, tag="o")
                        nc.vector.tensor_tensor(
                            o_t[:],
                            ps[:],
                            ob_t[:, jt : jt + 1].to_broadcast((P, HALF)),
                            mybir.AluOpType.add,
                        )
                        nc.sync.dma_start(
                            outT[jt * P : (jt + 1) * P, bsl], o_t[:]
                        )

    return nc


# ------------------------------------------------------------- host helpers
def _host_prepare(inputs):
    x = np.asarray(inputs["x"], dtype=np.float32)
    proj_w = np.asarray(inputs["proj_w"], dtype=np.float32)
    proj_b = np.asarray(inputs["proj_b"], dtype=np.float32)
    mix_w = np.asarray(inputs["mix_w"], dtype=np.float32)
    mix_b = np.asarray(inputs["mix_b"], dtype=np.float32)
    decay_value = np.asarray(inputs["decay_value"], dtype=np.float32)
    cache = np.asarray(inputs["cache"], dtype=np.float32)
    out_w = np.asarray(inputs["out_w"], dtype=np.float32)
    out_b = np.asarray(inputs["out_b"], dtype=np.float32)
    idx = int(np.asarray(inputs["index"]))

    w = mix_w[:, idx]  # [16]
    bb = mix_b[:, idx]  # [16]
    decay = np.clip(decay_value, 0.9, 1.0) ** np.float32(1.0 / DECAY_CONSTANT)
    is_col = np.arange(N_HEADS) < (N_HEADS // 2)
    coef = np.where(is_col, w * decay, decay).astype(np.float32)  # [16]

    # C[j, d] = out_w @ (w*proj_w reshaped [i, d]) : the x-path weights
    PW = (proj_w * w[:, None, None]).reshape(DIM, DIM)  # [i, d]
    C = out_w @ PW  # [j, d]

    # ob2[j] = out_b + out_w @ (w*proj_b + bb)
    bias_i = (w[:, None] * proj_b + bb[:, None]).reshape(DIM)  # [i]
    ob2 = out_b + out_w @ bias_i  # [j]

    # Wcat [K=8192, j]: rows 0..4095 = C.T, rows 4096.. = out_w.T
    Wcat = np.empty((KDIM, DIM), dtype=NP_BF16)
    Wcat[:DIM] = C.T.astype(NP_BF16)
    Wcat[DIM:] = out_w.T.astype(NP_BF16)
    # tile to [jt, p(k), kt, j]
    WT = np.ascontiguousarray(
        Wcat.reshape(KT, P, JT, P).transpose(2, 1, 0, 3)
    )

    # A.T [K=8192, batch]: rows 0..4095 = x.T, rows 4096.. = (coef*cache).T
    AT = np.empty((KDIM, BATCH), dtype=NP_BF16)
    AT[:DIM] = x.T.astype(NP_BF16)
    cc = (cache * coef[:, None, None]).astype(NP_BF16)  # [h, b, k]
    AT[DIM:] = cc.transpose(0, 2, 1).reshape(DIM, BATCH)

    obT = np.ascontiguousarray(ob2.reshape(JT, P).T)  # [P, JT]

    in_maps = []
    for c in range(N_CORES):
        bsl = slice(c * BC, (c + 1) * BC)
        in_maps.append(
            {
                "a": np.ascontiguousarray(AT[:, bsl]).reshape(KT, P, BC),
                "wt": WT,
                "ob": obT,
            }
        )
    return in_maps


def _assemble(results):
    # results: list per core of {"outT": [DIM, BC]}
    out = np.empty((BATCH, DIM), dtype=np.float32)
    for c in range(N_CORES):
        out[c * BC : (c + 1) * BC] = results[c]["outT"].T
    return out


_NC_CACHE = None


def _get_nc():
    global _NC_CACHE
    if _NC_CACHE is None:
        _NC_CACHE = build_kernel()
    return _NC_CACHE


def kernel(**inputs) -> np.ndarray:
    in_maps = _host_prepare(inputs)
    nc = _get_nc()
    res = run_bass_kernel_spmd(nc, in_maps, list(range(N_CORES)))
    return _assemble(res.results)


if __name__ == "__main__":
    # quick self-run with random data of the right shapes
    rng = np.random.default_rng(0)
    ins = {
        "x": rng.standard_normal((BATCH, DIM), dtype=np.float32),
        "proj_w": rng.standard_normal((N_HEADS, HIDDEN, DIM), dtype=np.float32) * 0.02,
        "proj_b": rng.standard_normal((N_HEADS, HIDDEN), dtype=np.float32) * 0.02,
        "mix_w": rng.standard_normal((N_HEADS, 4096), dtype=np.float32) * 0.02 + 1.0,
        "mix_b": rng.standard_normal((N_HEADS, 4096), dtype=np.float32) * 0.02,
        "decay_value": rng.uniform(0.85, 1.05, size=(N_HEADS,)).astype(np.float32),
        "cache": rng.standard_normal((N_HEADS, BATCH, HIDDEN), dtype=np.float32),
        "out_w": rng.standard_normal((DIM, DIM), dtype=np.float32) * 0.02,
        "out_b": rng.standard_normal((DIM,), dtype=np.float32) * 0.02,
        "index": 1000,
    }
    out = kernel(**ins)
    print("out", out.shape, out.dtype, float(np.abs(out).mean()))


# revision 3
# speedup vs baseline: 1.1081x; 1.0784x over previous
"""Trainium2 Bass kernel for nn_MixedRepeatHeads.

Computation (full shapes):
  proj[h,b,k] = einsum(x[b,d], proj_w[h,k,d]) + proj_b[h,k]
  w = mix_w[:, index]; bb = mix_b[:, index]
  decay = clip(decay_value, 0.9, 1.0) ** (1/8)
  coef[h] = w*decay (h<8) else decay
  hidden[b, h*256+k] = w[h]*proj[h,b,k] + coef[h]*cache[h,b,k] + bb[h]
  out = hidden @ out_w.T + out_b                     # [8192, 4096]

Algebraic refold (host side, cheap):
  out = x @ C.T + (coef*cache) @ out_w.T + ob2
    C   = out_w @ (w*proj_w)          (one 4096^3 host GEMM)
    ob2 = out_b + out_w @ (w*proj_b + bb)
  => ONE GEMM with K=8192:  out = A @ Wcat + ob2
    A    = [x | coef*cache]           # [8192, 8192]
    Wcat = [C.T ; out_w.T]            # [8192, 4096]

Device: data-parallel over batch across 8 cores (1024 rows/core), all-bf16
matmuls (N=512 moving, fp32 PSUM accumulation; bf16 hides PE weight loads,
unlike f32r whose matmuls must self-load serially).  The batch is processed
as two 512-column halves, each a full pass over the 32 j-tiles streaming the
weights; each half's activation tile reload (WAR-ordered by the tile pool)
overlaps the opposite half's entire compute pass, so steady-state has no
activation stalls and DMA queues never head-of-line block the weight stream.
All DRAM layouts are partition-major so every DMA descriptor is a large
contiguous run per partition.
Device output is outT [4096, 1024] f32 per core; host transposes + concats.
"""

import sys

if "/opt/trn_rl_repo" not in sys.path:
    sys.path.insert(0, "/opt/trn_rl_repo")

import contextlib

import numpy as np
import ml_dtypes

import bass_rust
import concourse.bass as bass
import concourse.tile as tile
from concourse import mybir
from concourse.bass_utils import run_bass_kernel_spmd
from concourse.vector_clock import ScopedClock

# ---------------------------------------------------------------- constants
N_HEADS = 16
HIDDEN = 256
DIM = 4096
BATCH = 8192
DECAY_CONSTANT = 8
N_CORES = 8
BC = BATCH // N_CORES  # 1024 batch rows per core
HALF = BC // 2  # 512
P = 128
KDIM = 2 * DIM  # 8192 contraction (x ++ cache)
KT = KDIM // P  # 64 k-tiles
JT = DIM // P  # 32 j-tiles

F32 = mybir.dt.float32
BF16 = mybir.dt.bfloat16
NP_BF16 = ml_dtypes.bfloat16

# ------------------------------------------------- walrus wait legalization
# This walrus build supports only ONE sync-wait command per instruction.
MAXW = 1


class SafeTileContext(tile.TileContext):
    def _split_waits_in_ordered(self, ordered):
        nc = self.nc
        for _bb_name, insts in ordered.items():
            new_list = []
            changed = False
            for inst in insts:
                si = inst.sync_info
                if si is not None and len(si.on_wait) > MAXW:
                    waits = list(si.on_wait)
                    ups = list(si.on_update)
                    head, tail = waits[:-MAXW], waits[-MAXW:]
                    for w in head:
                        nop = mybir.InstNoOp(
                            name=nc.get_next_instruction_name(),
                            engine=inst.engine,
                            ins=[],
                            outs=[],
                            sync_info=bass_rust.SyncInfo(on_wait=[w], on_update=[]),
                            bass_nofuse=True,
                        )
                        nc.register_instruction(nop, overwrite=True)
                        new_list.append(nop)
                    inst.sync_info = bass_rust.SyncInfo(on_wait=tail, on_update=ups)
                    changed = True
                new_list.append(inst)
            if changed:
                insts[:] = new_list
        return ordered

    def _lower_ordered_insts(self, ordered):
        self._split_waits_in_ordered(ordered)
        return super()._lower_ordered_insts(ordered)

    def _drain_and_barrier(self, tick_clock, wait_clock):
        probe = self.nc.sync.nop(nofuse=True)
        wait_clock.add_sem_waits(
            probe.ins, ScopedClock({None: tick_clock.global_clock})
        )
        si = probe.ins.sync_info
        waits = list(si.on_wait) if si is not None else []
        upd = list(si.on_update) if si is not None else []
        probe.ins.sync_info = bass_rust.SyncInfo(on_wait=waits[:MAXW], on_update=upd)
        for i in range(MAXW, len(waits), MAXW):
            n = self.nc.sync.nop(nofuse=True)
            n.ins.sync_info = bass_rust.SyncInfo(
                on_wait=waits[i : i + MAXW], on_update=[]
            )

        self.nc.sync.drain()

        self.nc.all_engine_barrier()
        assert self.sems is not None
        popped = self.nc._tile_sem_poison_stack.pop()
        assert popped is self._sem_poison
        self.nc.clear_and_free_semaphores(list(self.sems.allocated().values()))
        self.nc.all_engine_barrier()


# ------------------------------------------------------------ kernel build
def build_kernel(loop_reps=None):
    """Per-core program. DRAM params:
      a    [2, P, KT, HALF] bf16 : a[h, p, kt, b] = A[h*512+b, kt*128+p]
      wt   [JT, P, KT, P]   bf16 : wt[jt, p, kt, j] = Wcat[kt*128+p, jt*128+j]
      ob   [P, JT]          f32  : ob[p, jt] = ob2[jt*128+p]
      outT [DIM, BC]        f32  : output, transposed (j-major)
    """
    nc = bass.Bass()
    a = nc.declare_dram_parameter("a", [2, P, KT, HALF], BF16, isOutput=False)
    wt = nc.declare_dram_parameter("wt", [JT, P, KT, P], BF16, isOutput=False)
    ob = nc.declare_dram_parameter("ob", [P, JT], F32, isOutput=False)
    outT = nc.declare_dram_parameter("outT", [DIM, BC], F32, isOutput=True)

    ACH = 8  # a-half DMA chunks (8 kt per chunk, contiguous per partition)
    KCH = KT // ACH

    with SafeTileContext(nc) as tc:
        with (
            tc.tile_pool(name="a0pool", bufs=1) as a0pool,
            tc.tile_pool(name="a1pool", bufs=1) as a1pool,
            tc.tile_pool(name="wpool", bufs=3) as wpool,
            tc.tile_pool(name="opool", bufs=3) as opool,
            tc.tile_pool(name="obpool", bufs=1) as obpool,
            tc.tile_pool(name="pspool", bufs=6, space="PSUM") as pspool,
        ):
            ob_t = obpool.tile([P, JT], F32)
            nc.sync.dma_start(ob_t[:], ob[:])

            loop_cm = (
                tc.For_i(0, loop_reps, 1)
                if loop_reps is not None
                else contextlib.nullcontext()
            )
            with loop_cm:
                for half in range(2):
                    apool = a0pool if half == 0 else a1pool
                    a_t = apool.tile([P, KT, HALF], BF16, tag=f"a{half}")
                    for c in range(ACH):
                        ksl = slice(c * KCH, (c + 1) * KCH)
                        nc.sync.dma_start(a_t[:, ksl, :], a[half][:, ksl, :])
                    for jt in range(JT):
                        wblk = wpool.tile([P, KT, P], BF16, tag="wblk")
                        nc.sync.dma_start(wblk[:], wt[jt])
                        ps = pspool.tile([P, HALF], F32, tag="ps")
                        for kt in range(KT):
                            nc.tensor.matmul(
                                ps[:],
                                wblk[:, kt, :],
                                a_t[:, kt, :],
                                start=(kt == 0),
                                stop=(kt == KT - 1),
                            )
                        o_t = opool.tile([P, HALF], F32, tag="o")
                        nc.vector.tensor_tensor(
                            o_t[:],
                            ps[:],
                            ob_t[:, jt : jt + 1].to_broadcast((P, HALF)),
                            mybir.AluOpType.add,
                        )
                        nc.sync.dma_start(
                            outT[
                                jt * P : (jt + 1) * P,
                                half * HALF : (half + 1) * HALF,
                            ],
                            o_t[:],
                        )

    return nc


# ------------------------------------------------------------- host helpers
def _host_prepare(inputs):
    x = np.asarray(inputs["x"], dtype=np.float32)
    proj_w = np.asarray(inputs["proj_w"], dtype=np.float32)
    proj_b = np.asarray(inputs["proj_b"], dtype=np.float32)
    mix_w = np.asarray(inputs["mix_w"], dtype=np.float32)
    mix_b = np.asarray(inputs["mix_b"], dtype=np.float32)
    decay_value = np.asarray(inputs["decay_value"], dtype=np.float32)
    cache = np.asarray(inputs["cache"], dtype=np.float32)
    out_w = np.asarray(inputs["out_w"], dtype=np.float32)
    out_b = np.asarray(inputs["out_b"], dtype=np.float32)
    idx = int(np.asarray(inputs["index"]))

    w = mix_w[:, idx]  # [16]
    bb = mix_b[:, idx]  # [16]
    decay = np.clip(decay_value, 0.9, 1.0) ** np.float32(1.0 / DECAY_CONSTANT)
    is_col = np.arange(N_HEADS) < (N_HEADS // 2)
    coef = np.where(is_col, w * decay, decay).astype(np.float32)  # [16]

    PW = (proj_w * w[:, None, None]).reshape(DIM, DIM)  # [i, d]
    C = out_w @ PW  # [j, d]
    bias_i = (w[:, None] * proj_b + bb[:, None]).reshape(DIM)  # [i]
    ob2 = out_b + out_w @ bias_i  # [j]

    # Wcat [K=8192, j]: rows 0..4095 = C.T, rows 4096.. = out_w.T
    Wcat = np.empty((KDIM, DIM), dtype=NP_BF16)
    Wcat[:DIM] = C.T.astype(NP_BF16)
    Wcat[DIM:] = out_w.T.astype(NP_BF16)
    WT = np.ascontiguousarray(Wcat.reshape(KT, P, JT, P).transpose(2, 1, 0, 3))

    # A.T [K=8192, batch]: rows 0..4095 = x.T, rows 4096.. = (coef*cache).T
    AT = np.empty((KDIM, BATCH), dtype=NP_BF16)
    AT[:DIM] = x.T.astype(NP_BF16)
    cc = (cache * coef[:, None, None]).astype(NP_BF16)  # [h, b, k]
    AT[DIM:] = cc.transpose(0, 2, 1).reshape(DIM, BATCH)
    # [kt, p, core, half, b'] -> per-core [2, P, KT, HALF] (partition-major)
    AT_r = AT.reshape(KT, P, N_CORES, 2, HALF).transpose(2, 3, 1, 0, 4)

    obT = np.ascontiguousarray(ob2.reshape(JT, P).T)  # [P, JT]

    in_maps = []
    for c in range(N_CORES):
        in_maps.append(
            {
                "a": np.ascontiguousarray(AT_r[c]),
                "wt": WT,
                "ob": obT,
            }
        )
    return in_maps


def _assemble(results):
    # results: list per core of {"outT": [DIM, BC]}
    out = np.empty((BATCH, DIM), dtype=np.float32)
    for c in range(N_CORES):
        out[c * BC : (c + 1) * BC] = results[c]["outT"].T
    return out


_NC_CACHE = None


def _get_nc():
    global _NC_CACHE
    if _NC_CACHE is None:
        _NC_CACHE = build_kernel()
    return _NC_CACHE


def kernel(**inputs) -> np.ndarray:
    in_maps = _host_prepare(inputs)
    nc = _get_nc()
    res = run_bass_kernel_spmd(nc, in_maps, list(range(N_CORES)))
    return _assemble(res.results)


if __name__ == "__main__":
    # quick self-run with random data of the right shapes
    rng = np.random.default_rng(0)
    ins = {
        "x": rng.standard_normal((BATCH, DIM), dtype=np.float32),
        "proj_w": rng.standard_normal((N_HEADS, HIDDEN, DIM), dtype=np.float32) * 0.02,
        "proj_b": rng.standard_normal((N_HEADS, HIDDEN), dtype=np.float32) * 0.02,
        "mix_w": rng.standard_normal((N_HEADS, 4096), dtype=np.float32) * 0.02 + 1.0,
        "mix_b": rng.standard_normal((N_HEADS, 4096), dtype=np.float32) * 0.02,
        "decay_value": rng.uniform(0.85, 1.05, size=(N_HEADS,)).astype(np.float32),
        "cache": rng.standard_normal((N_HEADS, BATCH, HIDDEN), dtype=np.float32),
        "out_w": rng.standard_normal((DIM, DIM), dtype=np.float32) * 0.02,
        "out_b": rng.standard_normal((DIM,), dtype=np.float32) * 0.02,
        "index": 1000,
    }
    out = kernel(**ins)
    print("out", out.shape, out.dtype, float(np.abs(out).mean()))
